# revision 16
# baseline (speedup 1.0000x reference)
"""Distributed Trainium2 Bass kernel for the 3-layer GNN message-passing model.

Strategy (8 NeuronCores):
- Nodes are partitioned into 8 contiguous ranges of NPC (N padded up);
  each core owns all edges whose *dst* lands in its range, so the segment-sum
  aggregation is fully core-local (no reduction collective).
- The message-MLP first layer is decomposed:
      inp @ msg_w1 = Hd[dst] + Hs[src] + edge_attr @ (edge_w @ W1e) + b1'
  with node tables Hd = h@W1[:H], Hs = h@W1[H:2H].  Hd is only needed for
  local dst nodes; Hs is AllGather'ed once per layer (the only collective).
- Edges are sorted by BLK-node dst block.  Per TILE_E-edge tile, selection
  matrices (is_equal vs iota) turn the dst-table gather and the scatter-add
  into TensorE matmuls accumulating in PSUM; src rows come via grouped
  indirect DMA gathers from the AllGather'ed table.
- Activations stay feature-major so every matmul contracts on partitions with
  no layout changes; fp32 matmuls stream at full rate via float32r.
"""

import sys

if "/opt/trn_rl_repo" not in sys.path:
    sys.path.insert(0, "/opt/trn_rl_repo")

import numpy as np

from concourse import bass, bacc, bass_utils, tile, mybir

F32 = mybir.dt.float32
F32R = mybir.dt.float32r
I32 = mybir.dt.int32

P = 128          # partitions
PADV = 300.0     # dstrel padding value (matches no window index)


class Cfg:
    def __init__(self, n_nodes=50000, n_edges=400000, node_in=128, edge_dim=32,
                 hidden=256, n_layers=3, n_cores=8, npc=6400, blk=256,
                 tile_e=256, group_tiles=4):
        self.N = n_nodes
        self.E = n_edges
        self.NODE_IN = node_in
        self.EDGE_DIM = edge_dim
        self.H = hidden
        self.L = n_layers
        self.C = n_cores
        self.NPC = npc                  # nodes per core (padded)
        self.BLK = blk                  # dst node window per block
        self.TILE_E = tile_e            # edges per tile
        self.GT = group_tiles           # tiles per indirect-gather group
        assert npc * n_cores >= n_nodes
        assert hidden == 256 and node_in == 128 and tile_e == 256 and blk == 256
        assert npc % blk == 0 and (npc // P) % 2 == 0
        self.NBLK = npc // blk
        self.NT = npc // P              # 128-row node tiles per core
        self.FC = hidden // P           # feature chunks (2)
        # node-column groups for dense phases
        self.NGRP = [(s, min(512, npc - s)) for s in range(0, npc, 512)]


# ----------------------------------------------------------------------------
# host-side preprocessing
# ----------------------------------------------------------------------------

def _preprocess(cfg, edge_index, edge_attr):
    """Shard edges by dst owner, sort by (dst block, src), split each block at
    the int16 half-table boundary, pad per-(block,half) to cross-core-uniform
    tile counts, and build the per-core device streams."""
    src = np.asarray(edge_index[0], dtype=np.int64)
    dst = np.asarray(edge_index[1], dtype=np.int64)
    ea = np.asarray(edge_attr, dtype=np.float32)
    C, NPC, BLK, TE, NBLK, GT = cfg.C, cfg.NPC, cfg.BLK, cfg.TILE_E, cfg.NBLK, cfg.GT
    HB = 32768                       # int16 gather half-table boundary

    owner = dst // NPC
    per_core = []
    cnt0 = np.zeros((C, NBLK), dtype=np.int64)
    cnt1 = np.zeros((C, NBLK), dtype=np.int64)
    for c in range(C):
        m = owner == c
        s_c, d_c, ea_c = src[m], dst[m], ea[m]
        blk_c = (d_c - c * NPC) // BLK
        order = np.lexsort((s_c, blk_c))
        s_c, d_c, ea_c, blk_c = s_c[order], d_c[order], ea_c[order], blk_c[order]
        per_core.append((s_c, d_c, ea_c))
        h1 = (s_c >= HB).astype(np.int64)
        cnt0[c] = np.bincount(blk_c, weights=1 - h1, minlength=NBLK).astype(np.int64)
        cnt1[c] = np.bincount(blk_c, weights=h1, minlength=NBLK).astype(np.int64)

    Kb0 = (-(-cnt0 // TE)).max(axis=0)
    Kb1 = (-(-cnt1 // TE)).max(axis=0)
    Kb0 = np.maximum(Kb0, (Kb0 + Kb1) == 0)          # >=1 tile per block
    Kb = (Kb0 + Kb1).astype(int)
    T_total = int(Kb.sum())
    EL = T_total * TE
    starts = np.concatenate([[0], np.cumsum(Kb * TE)]).astype(np.int64)

    # gather groups: runs of <=GT tiles within one (block, half)
    groups = []
    t = 0
    for b in range(NBLK):
        for half, K in ((0, int(Kb0[b])), (1, int(Kb1[b]))):
            for off in range(0, K, GT):
                groups.append({"t0": t + off, "ng": min(GT, K - off), "half": half})
            t += K

    streams = []
    for c in range(C):
        s_c, d_c, ea_c = per_core[c]
        srcL = np.zeros(EL, dtype=np.int64)
        drlL = np.full(EL, PADV, dtype=np.float32)
        eaL = np.zeros((EL, cfg.EDGE_DIM), dtype=np.float32)
        n0 = np.concatenate([[0], np.cumsum(cnt0[c] + cnt1[c])]).astype(np.int64)
        for b in range(NBLK):
            for half, base_slot, n in ((0, starts[b], int(cnt0[c][b])),
                                       (1, starts[b] + int(Kb0[b]) * TE,
                                        int(cnt1[c][b]))):
                if n == 0:
                    continue
                i0 = n0[b] + (int(cnt0[c][b]) if half else 0)
                sl_in = slice(i0, i0 + n)
                sl_out = slice(base_slot, base_slot + n)
                srcL[sl_out] = s_c[sl_in] - half * HB
                drlL[sl_out] = (d_c[sl_in] - (c * NPC + b * BLK)).astype(np.float32)
                eaL[sl_out] = ea_c[sl_in]
        # idx16: per tile 16 columns, element i of a tile at [i % 16, i // 16]
        idx16 = srcL.astype(np.int16).reshape(T_total, 16, 16)
        idx16 = np.ascontiguousarray(
            np.tile(idx16.transpose(2, 0, 1).reshape(16, T_total * 16), (8, 1)))
        streams.append({
            "idx16": idx16,
            "dstrel": np.ascontiguousarray(drlL.reshape(-1, P).T),
            "eaT": np.ascontiguousarray(eaL.T),
        })

    return {"Kb": Kb.tolist(), "Kb0": Kb0.astype(int).tolist(),
            "Kb1": Kb1.astype(int).tolist(), "groups": groups, "HB": HB,
            "T_total": T_total, "EL": EL, "streams": streams}


def _pack_weights(cfg, inp):
    """Host-side weight packing (replicated to every core)."""
    H, FC = cfg.H, cfg.FC
    w = {}
    ew = np.asarray(inp["edge_w"], np.float32)
    eb = np.asarray(inp["edge_b"], np.float32)
    for l in range(cfg.L):
        W1 = np.asarray(inp["msg_w1"][l], np.float32)
        W1d, W1s, W1e = W1[:H], W1[H:2 * H], W1[2 * H:]
        b1 = np.asarray(inp["msg_b1"][l], np.float32)
        w[f"w1d_{l}"] = np.hstack([W1d[kc * P:(kc + 1) * P] for kc in range(FC)])
        w[f"w1s_{l}"] = np.hstack([W1s[kc * P:(kc + 1) * P] for kc in range(FC)])
        w[f"w1e_{l}"] = (ew @ W1e).astype(np.float32)                  # [32, H]
        b1p = b1 + eb @ W1e
        w[f"b1_{l}"] = np.stack([b1p[fc * P:(fc + 1) * P] for fc in range(FC)], 1)
        W2 = np.asarray(inp["msg_w2"][l], np.float32)
        w[f"w2_{l}"] = np.hstack([W2[kc * P:(kc + 1) * P] for kc in range(FC)])
        w[f"b2_{l}"] = np.asarray(inp["msg_b2"][l], np.float32)[None, :]
        U1 = np.asarray(inp["up_w1"][l], np.float32)
        w[f"up1_{l}"] = np.hstack([U1[kc * P:(kc + 1) * P, fc * P:(fc + 1) * P]
                                   for kc in range(4) for fc in range(FC)])
        ub1 = np.asarray(inp["up_b1"][l], np.float32)
        w[f"ub1_{l}"] = np.stack([ub1[fc * P:(fc + 1) * P] for fc in range(FC)], 1)
        U2 = np.asarray(inp["up_w2"][l], np.float32)
        w[f"up2_{l}"] = np.hstack([U2[kc * P:(kc + 1) * P, fc * P:(fc + 1) * P]
                                   for kc in range(FC) for fc in range(FC)])
        ub2 = np.asarray(inp["up_b2"][l], np.float32)
        w[f"ub2_{l}"] = np.stack([ub2[fc * P:(fc + 1) * P] for fc in range(FC)], 1)
    w["node_w"] = np.asarray(inp["node_w"], np.float32)                # [128, H]
    nb = np.asarray(inp["node_b"], np.float32)
    w["node_b"] = np.stack([nb[fc * P:(fc + 1) * P] for fc in range(FC)], 1)
    T1 = np.asarray(inp["tok_w1"], np.float32)                         # [H, H/2]
    w["tok1"] = np.hstack([T1[kc * P:(kc + 1) * P] for kc in range(FC)])
    w["tokb1"] = np.asarray(inp["tok_b1"], np.float32)[:, None]
    w["tok2"] = np.asarray(inp["tok_w2"], np.float32)                  # [H/2, 1]
    w["tokb2"] = np.asarray(inp["tok_b2"], np.float32).reshape(1, 1)
    w["iota_row"] = np.broadcast_to(
        np.arange(cfg.BLK, dtype=np.float32)[None, :], (P, cfg.BLK)).copy()
    w["iota_c0"] = np.arange(P, dtype=np.float32)[:, None].copy()
    w["iota_c1"] = w["iota_c0"] + P
    w["ident"] = np.eye(P, dtype=np.float32)
    w["ones1"] = np.ones((1, P), dtype=np.float32)
    return w


def _weight_shapes(cfg):
    """name -> (shape, dtype).  Tensors feeding fp32r matmuls are float32r."""
    H, FC, ED = cfg.H, cfg.FC, cfg.EDGE_DIM
    shapes = {}
    for l in range(cfg.L):
        shapes.update({
            f"w1d_{l}": ([P, FC * H], F32R), f"w1s_{l}": ([P, FC * H], F32R),
            f"w1e_{l}": ([ED, H], F32R), f"b1_{l}": ([P, FC], F32),
            f"w2_{l}": ([P, FC * H], F32R), f"b2_{l}": ([1, H], F32R),
            f"up1_{l}": ([P, 8 * P], F32R), f"ub1_{l}": ([P, FC], F32),
            f"up2_{l}": ([P, 4 * P], F32R), f"ub2_{l}": ([P, FC], F32),
        })
    shapes.update({
        "node_w": ([P, H], F32R), "node_b": ([P, FC], F32),
        "tok1": ([P, FC * P], F32R), "tokb1": ([P, 1], F32),
        "tok2": ([P, 1], F32R), "tokb2": ([1, 1], F32),
        "iota_row": ([P, cfg.BLK], F32), "iota_c0": ([P, 1], F32),
        "iota_c1": ([P, 1], F32), "ident": ([P, P], F32),
        "ones1": ([1, P], F32R),
    })
    return shapes


# ----------------------------------------------------------------------------
# device graph
# ----------------------------------------------------------------------------

def _build(cfg, prep):
    H, FC, NPC, BLK, TE, GT = cfg.H, cfg.FC, cfg.NPC, cfg.BLK, cfg.TILE_E, cfg.GT
    NBLK, NT, L, C = cfg.NBLK, cfg.NT, cfg.L, cfg.C
    Kb, T_total, EL = prep["Kb"], prep["T_total"], prep["EL"]
    Kb0, groups, HB = prep["Kb0"], prep["groups"], prep["HB"]
    NTAB = NPC * C
    Relu = mybir.ActivationFunctionType.Relu
    Copy = mybir.ActivationFunctionType.Copy
    EQ = mybir.AluOpType.is_equal

    nc = bacc.Bacc(None, target_bir_lowering=False, debug=False, num_devices=C)

    def din(name, shape, dt=F32):
        return nc.dram_tensor(name, shape, dt, kind="ExternalInput")

    xT_d = din("xT", [cfg.NODE_IN, NPC], F32R)
    idx16_d = din("idx16", [P, 16 * T_total], mybir.dt.int16)
    dstrel_d = din("dstrel", [P, 2 * T_total])
    eaT_d = din("eaT", [cfg.EDGE_DIM, EL], F32R)
    wtensors = {nm: din(nm, shp, dt) for nm, (shp, dt) in _weight_shapes(cfg).items()}

    hT_out = nc.dram_tensor("hT_out", [H, NPC], F32R, kind="ExternalOutput")
    lg_out = nc.dram_tensor("lg_out", [1, NPC], F32, kind="ExternalOutput")

    with tile.TileContext(nc) as tc:
        with (
            tc.tile_pool(name="const", bufs=1) as cp,
            tc.tile_pool(name="persist", bufs=1) as pe,
            tc.tile_pool(name="work", bufs=2) as wk,
            tc.tile_pool(name="psum", bufs=1, space="PSUM") as pp,
            tc.tile_pool(name="dram", bufs=1, space="DRAM") as dr,
        ):
            # ---- constants & weights --------------------------------------
            W = {}
            for nm, d in wtensors.items():
                t = cp.tile(list(d.shape), d.dtype, name=f"c_{nm}")
                nc.sync.dma_start(out=t[:], in_=d.ap())
                W[nm] = t
            idx16 = cp.tile([P, 16 * T_total], mybir.dt.int16, name="idx16_sb")
            nc.sync.dma_start(out=idx16[:], in_=idx16_d.ap())
            dstrel = cp.tile([P, 2 * T_total], F32, name="dstrel_sb")
            nc.sync.dma_start(out=dstrel[:], in_=dstrel_d.ap())

            # ---- persistent activations (feature-major) -------------------
            hT = [pe.tile([P, NPC], F32R, name=f"hT{fc}") for fc in range(FC)]

            # ---- internal DRAM --------------------------------------------
            hd_dram = dr.tile([NPC, H], F32R, name="hd_dram", bufs=2)
            ag_dram = dr.tile([FC, P, NPC], F32R, name="ag_dram", bufs=2)
            hs_in = dr.tile([NPC, H], F32, name="hs_in", bufs=2)

            # ---- phase 0: hT = (x @ node_w + node_b)^T --------------------
            for (s0, wd) in cfg.NGRP:
                cs = slice(s0, s0 + wd)
                xt = wk.tile([P, 512], F32R, name="xt", tag="xt", bufs=2)
                nc.sync.dma_start(out=xt[:, :wd], in_=xT_d.ap()[:, cs])
                for fc in range(FC):
                    ps = pp.tile([P, 512], F32, space="PSUM", name="ph_ps",
                                 tag="a", bufs=2)
                    nc.tensor.matmul(
                        out=ps[:, :wd],
                        lhsT=W["node_w"][:, fc * P:(fc + 1) * P],
                        rhs=xt[:, :wd], start=True, stop=True)
                    nc.vector.tensor_tensor(
                        out=hT[fc][:, cs], in0=ps[:, :wd],
                        in1=W["node_b"][:, fc:fc + 1].to_broadcast([P, wd]),
                        op=mybir.AluOpType.add)

            # ---- layers ----------------------------------------------------
            for l in range(L):
                hs_tab = dr.tile([NTAB, H], F32, name=f"hs_tab_{l}", bufs=2,
                                 addr_space="Shared")
                # Hs (node-major) -> hs_in --AllGather--> hs_tab
                for nt2 in range(NT // 2):
                    stg = wk.tile([P, 2, H], F32, name="stg", tag="stg", bufs=3)
                    for j in range(2):
                        ntl = nt2 * 2 + j
                        ps = pp.tile([P, H], F32, space="PSUM", name="hs_ps",
                                     tag="drT", bufs=2)
                        for kc in range(FC):
                            nc.tensor.matmul(
                                out=ps[:],
                                lhsT=hT[kc][:, ntl * P:(ntl + 1) * P],
                                rhs=W[f"w1s_{l}"][:, kc * H:(kc + 1) * H],
                                start=(kc == 0), stop=(kc == FC - 1))
                        nc.vector.tensor_copy(out=stg[:, j], in_=ps[:])
                    nc.sync.dma_start(
                        out=hs_in[nt2 * 2 * P:(nt2 + 1) * 2 * P, :]
                            .rearrange("(j p) d -> p j d", p=P),
                        in_=stg[:])
                nc.gpsimd.collective_compute(
                    "AllGather", mybir.AluOpType.bypass,
                    replica_groups=[list(range(C))],
                    ins=[hs_in[:].opt()], outs=[hs_tab[:].opt()])

                # Hd (node-major) -> hd_dram (overlaps the AllGather)
                for nt2 in range(NT // 2):
                    stg2 = wk.tile([P, 2, H], F32R, name="stg2", tag="stg", bufs=3)
                    for j in range(2):
                        ntl = nt2 * 2 + j
                        ps = pp.tile([P, H], F32, space="PSUM", name="hd_ps",
                                     tag="drT", bufs=2)
                        for kc in range(FC):
                            nc.tensor.matmul(
                                out=ps[:],
                                lhsT=hT[kc][:, ntl * P:(ntl + 1) * P],
                                rhs=W[f"w1d_{l}"][:, kc * H:(kc + 1) * H],
                                start=(kc == 0), stop=(kc == FC - 1))
                        nc.vector.tensor_copy(out=stg2[:, j], in_=ps[:])
                    nc.sync.dma_start(
                        out=hd_dram[nt2 * 2 * P:(nt2 + 1) * 2 * P, :]
                            .rearrange("(j p) d -> p j d", p=P),
                        in_=stg2[:])

                # ---- edge phase -------------------------------------------
                t_flat = 0
                gs_cur = ea_cur = None
                gs_base = 0
                gi = 0
                for b in range(NBLK):
                    hd_win = [wk.tile([P, H], F32R, name=f"hdw{ns}", tag=f"hdw{ns}",
                                      bufs=2) for ns in range(2)]
                    for ns in range(2):
                        nc.sync.dma_start(
                            out=hd_win[ns][:],
                            in_=hd_dram[(b * BLK + ns * P):(b * BLK + (ns + 1) * P), :])
                    ag_ps = [pp.tile([P, BLK], F32, space="PSUM", name=f"ag_ps{mfc}",
                                     tag=f"ag{mfc}", bufs=1) for mfc in range(FC)]
                    for k in range(Kb[b]):
                        t = t_flat + k
                        if gi < len(groups) and t == groups[gi]["t0"]:
                            g = groups[gi]
                            gi += 1
                            gs_base, ng = g["t0"], g["ng"]
                            tab_ap = (hs_tab[:min(HB, NTAB), :] if g["half"] == 0
                                      else hs_tab[HB:NTAB, :])
                            gs_cur = wk.tile([P, 2 * GT, H], F32, name="gs",
                                             tag="gs", bufs=2)
                            nc.gpsimd.dma_gather(
                                out_ap=gs_cur[:, :2 * ng, :],
                                in_ap=tab_ap,
                                idxs_ap=idx16[:, 16 * t:16 * (t + ng)],
                                num_idxs=ng * TE, num_idxs_reg=ng * TE,
                                elem_size=H)
                            ea_cur = wk.tile([cfg.EDGE_DIM, GT * TE], F32R,
                                             name="ea", tag="ea", bufs=2)
                            nc.sync.dma_start(
                                out=ea_cur[:, :ng * TE],
                                in_=eaT_d.ap()[:, t * TE:(t + ng) * TE])
                        tg = t - gs_base
                        # selection matrices
                        drT_ps = pp.tile([P, TE], F32, space="PSUM", name="drT_ps",
                                         tag="drT", bufs=2)
                        for es in range(2):
                            nc.tensor.transpose(
                                out=drT_ps[:, es * P:(es + 1) * P],
                                in_=dstrel[:, 2 * t + es:2 * t + es + 1]
                                    .to_broadcast([P, P]),
                                identity=W["ident"][:])
                        psT = [wk.tile([P, TE], F32R, name=f"psT{ns}", tag=f"psT{ns}",
                                       bufs=2) for ns in range(2)]
                        for ns in range(2):
                            nc.vector.tensor_tensor(
                                out=psT[ns][:], in0=drT_ps[:],
                                in1=W[f"iota_c{ns}"][:].to_broadcast([P, TE]), op=EQ)
                        psel = [wk.tile([P, BLK], F32R, name=f"psel{es}",
                                        tag=f"psel{es}", bufs=2) for es in range(2)]
                        for es in range(2):
                            nc.vector.tensor_tensor(
                                out=psel[es][:],
                                in0=dstrel[:, 2 * t + es:2 * t + es + 1]
                                    .to_broadcast([P, BLK]),
                                in1=W["iota_row"][:], op=EQ)
                        # a^T accumulation
                        a_ps = pp.tile([P, 2 * TE], F32, space="PSUM", name="a_ps",
                                       tag="a", bufs=2)
                        for fc in range(FC):
                            asl = slice(fc * TE, (fc + 1) * TE)
                            for ns in range(2):
                                nc.tensor.matmul(
                                    out=a_ps[:, asl],
                                    lhsT=hd_win[ns][:, fc * P:(fc + 1) * P],
                                    rhs=psT[ns][:],
                                    start=(ns == 0), stop=False)
                            for es in range(2):
                                nc.tensor.matmul(
                                    out=a_ps[:, fc * TE + es * P:fc * TE + (es + 1) * P],
                                    lhsT=gs_cur[:, 2 * tg + es, fc * P:(fc + 1) * P],
                                    rhs=W["ident"][:], is_transpose=True,
                                    start=False, stop=False)
                            nc.tensor.matmul(
                                out=a_ps[:, asl],
                                lhsT=W[f"w1e_{l}"][:, fc * P:(fc + 1) * P],
                                rhs=ea_cur[:, tg * TE:(tg + 1) * TE],
                                start=False, stop=True)
                        arelu = wk.tile([P, 2 * TE], F32R, name="arelu", tag="arelu",
                                        bufs=2)
                        for fc in range(FC):
                            asl = slice(fc * TE, (fc + 1) * TE)
                            nc.scalar.activation(out=arelu[:, asl], in_=a_ps[:, asl],
                                                 func=Relu,
                                                 bias=W[f"b1_{l}"][:, fc:fc + 1])
                        # m = relu(arelu @ W2 + b2)  (edge-major)
                        m_ps = pp.tile([P, 2 * H], F32, space="PSUM", name="m_ps",
                                       tag="m", bufs=2)
                        for es in range(2):
                            msl = slice(es * H, (es + 1) * H)
                            for fc in range(FC):
                                nc.tensor.matmul(
                                    out=m_ps[:, msl],
                                    lhsT=arelu[:, fc * TE + es * P:fc * TE + (es + 1) * P],
                                    rhs=W[f"w2_{l}"][:, fc * H:(fc + 1) * H],
                                    start=(fc == 0), stop=False)
                            nc.tensor.matmul(
                                out=m_ps[:, msl],
                                lhsT=W["ones1"][:],
                                rhs=W[f"b2_{l}"][:],
                                start=False, stop=True)
                        mrelu = wk.tile([P, 2 * H], F32R, name="mrelu", tag="mrelu",
                                        bufs=2)
                        for es in range(2):
                            msl = slice(es * H, (es + 1) * H)
                            nc.scalar.activation(out=mrelu[:, msl], in_=m_ps[:, msl],
                                                 func=Relu)
                        # scatter into aggr^T psum (accumulates over the block)
                        for mfc in range(FC):
                            for es in range(2):
                                nc.tensor.matmul(
                                    out=ag_ps[mfc][:],
                                    lhsT=mrelu[:, es * H + mfc * P:es * H + (mfc + 1) * P],
                                    rhs=psel[es][:],
                                    start=(k == 0 and es == 0),
                                    stop=(k == Kb[b] - 1 and es == 1))
                    t_flat += Kb[b]
                    agst = wk.tile([P, 2, BLK], F32R, name="agst", tag="agst", bufs=2)
                    for mfc in range(FC):
                        nc.vector.tensor_copy(out=agst[:, mfc], in_=ag_ps[mfc][:])
                    nc.sync.dma_start(
                        out=ag_dram[:, :, b * BLK:(b + 1) * BLK], in_=agst[:])

                # ---- update MLP (in place on hT) ---------------------------
                for (s0, wd) in cfg.NGRP:
                    cs = slice(s0, s0 + wd)
                    agld = wk.tile([P, 2, 512], F32R, name="agld", tag="agld", bufs=2)
                    nc.sync.dma_start(out=agld[:, :, :wd], in_=ag_dram[:, :, cs])
                    u1r = wk.tile([P, 2, 512], F32R, name="u1r", tag="u1r", bufs=2)
                    for fc in range(FC):
                        u1_ps = pp.tile([P, 512], F32, space="PSUM", name="u1_ps",
                                        tag="a", bufs=2)
                        for kc in range(4):
                            rhs = hT[kc][:, cs] if kc < 2 else agld[:, kc - 2, :wd]
                            nc.tensor.matmul(
                                out=u1_ps[:, :wd],
                                lhsT=W[f"up1_{l}"][:, (kc * FC + fc) * P:(kc * FC + fc + 1) * P],
                                rhs=rhs,
                                start=(kc == 0), stop=(kc == 3))
                        nc.scalar.activation(out=u1r[:, fc, :wd], in_=u1_ps[:, :wd],
                                             func=Relu, bias=W[f"ub1_{l}"][:, fc:fc + 1])
                    for fc in range(FC):
                        u2_ps = pp.tile([P, 512], F32, space="PSUM", name="u2_ps",
                                        tag="m", bufs=2)
                        for kc in range(FC):
                            nc.tensor.matmul(
                                out=u2_ps[:, :wd],
                                lhsT=W[f"up2_{l}"][:, (kc * FC + fc) * P:(kc * FC + fc + 1) * P],
                                rhs=u1r[:, kc, :wd],
                                start=(kc == 0), stop=(kc == FC - 1))
                        nc.scalar.activation(out=hT[fc][:, cs], in_=u2_ps[:, :wd],
                                             func=Relu, bias=W[f"ub2_{l}"][:, fc:fc + 1])

            # ---- token head -----------------------------------------------
            for (s0, wd) in cfg.NGRP:
                cs = slice(s0, s0 + wd)
                t1_ps = pp.tile([P, 512], F32, space="PSUM", name="t1_ps",
                                tag="a", bufs=2)
                for kc in range(FC):
                    nc.tensor.matmul(
                        out=t1_ps[:, :wd],
                        lhsT=W["tok1"][:, kc * P:(kc + 1) * P],
                        rhs=hT[kc][:, cs],
                        start=(kc == 0), stop=(kc == FC - 1))
                t1r = wk.tile([P, 512], F32R, name="t1r", tag="t1r", bufs=2)
                nc.scalar.activation(out=t1r[:, :wd], in_=t1_ps[:, :wd], func=Relu,
                                     bias=W["tokb1"][:])
                lg_ps = pp.tile([1, 512], F32, space="PSUM", name="lg_ps",
                                tag="m", bufs=2)
                nc.tensor.matmul(out=lg_ps[:, :wd], lhsT=W["tok2"][:],
                                 rhs=t1r[:, :wd], start=True, stop=True)
                lg_sb = wk.tile([1, 512], F32, name="lg_sb", tag="lg", bufs=2)
                nc.vector.tensor_tensor(
                    out=lg_sb[:, :wd], in0=lg_ps[:, :wd],
                    in1=W["tokb2"][:].to_broadcast([1, wd]),
                    op=mybir.AluOpType.add)
                nc.sync.dma_start(out=lg_out.ap()[:, cs], in_=lg_sb[:, :wd])

            # ---- outputs ---------------------------------------------------
            for fc in range(FC):
                nc.sync.dma_start(out=hT_out.ap()[fc * P:(fc + 1) * P, :],
                                  in_=hT[fc][:])

    nc.finalize()
    return nc


# ----------------------------------------------------------------------------
# entry point
# ----------------------------------------------------------------------------

def _make_in_maps(cfg, inputs, prep):
    wpack = _pack_weights(cfg, inputs)
    x = np.asarray(inputs["x"], np.float32)
    in_maps = []
    for c in range(cfg.C):
        st = prep["streams"][c]
        lo, hi = c * cfg.NPC, (c + 1) * cfg.NPC
        xs = np.zeros((cfg.NPC, cfg.NODE_IN), np.float32)
        real = max(0, min(hi, cfg.N) - lo)
        if real > 0:
            xs[:real] = x[lo:lo + real]
        m = {"xT": np.ascontiguousarray(xs.T), "idx16": st["idx16"],
             "dstrel": st["dstrel"], "eaT": st["eaT"]}
        m.update(wpack)
        in_maps.append(m)
    return in_maps


def _assemble(cfg, results):
    h = np.empty((cfg.C * cfg.NPC, cfg.H), np.float32)
    lg = np.empty(cfg.C * cfg.NPC, np.float32)
    for c in range(cfg.C):
        h[c * cfg.NPC:(c + 1) * cfg.NPC] = results[c]["hT_out"].T
        lg[c * cfg.NPC:(c + 1) * cfg.NPC] = results[c]["lg_out"][0]
    return lg[:cfg.N], h[:cfg.N]


def _run(cfg, inputs, trace=False):
    prep = _preprocess(cfg, inputs["edge_index"], inputs["edge_attr"])
    nc = _build(cfg, prep)
    in_maps = _make_in_maps(cfg, inputs, prep)
    res = bass_utils.run_bass_kernel_spmd(nc, in_maps, core_ids=list(range(cfg.C)),
                                          trace=trace)
    out = _assemble(cfg, [res.results[c] for c in range(cfg.C)])
    return out, res


def kernel(**inputs):
    cfg = Cfg()
    (lg, h), _ = _run(cfg, inputs)
    return lg, h
